# revision 48
# baseline (speedup 1.0000x reference)
"""Tensor-parallel causal multi-head attention on 8 Trainium2 NeuronCores.

Sharding: core c -> (batch b = c//2, head-group g = c%2 of 8 heads).
Each core computes its 8 heads' attention for its batch plus the partial
output projection (its heads' rows of W_out); the host sums the two
head-group partials per batch and adds the output bias.

All matmuls run in float32r (full PE rate, ~1e-3 rounding) with fp32 PSUM
accumulation. Softmax uses no max-subtraction (scores/8 are bounded ~|2|),
with denominators computed via an appended ones-column in V and applied
after the weighted sum.
"""

import numpy as np

B, T, DIN, DOUT, H, HD = 4, 2048, 1024, 1024, 16, 64
HPC = 8            # heads per core
DH = HPC * HD      # 512 head dims per core
NCORES = 8
TQC = 512          # tq chunk (matmul moving width)
NTQ = T // TQC     # 4
NTK = T // 128     # 16 tk tiles
ND = DIN // 128    # 8 contraction tiles

_built = None


def _build():
    import concourse.bacc as bacc
    import concourse.tile as tile
    from concourse import mybir

    f32 = mybir.dt.float32
    f32r = mybir.dt.float32r
    EXP = mybir.ActivationFunctionType.Exp

    nc = bacc.Bacc("TRN2", target_bir_lowering=False, debug=False, num_devices=NCORES)

    xT = nc.declare_dram_parameter("xT", [DIN, T], f32r, isOutput=False)
    wq = nc.declare_dram_parameter("wq", [DIN, DH], f32r, isOutput=False)
    wk = nc.declare_dram_parameter("wk", [DIN, DH], f32r, isOutput=False)
    wv = nc.declare_dram_parameter("wv", [DIN, DH], f32r, isOutput=False)
    wo = nc.declare_dram_parameter("wo", [DH, DOUT], f32r, isOutput=False)
    tri = nc.declare_dram_parameter("tri", [128, 128], f32r, isOutput=False)
    out = nc.declare_dram_parameter("out", [T, DOUT], f32, isOutput=True)

    with tile.TileContext(nc) as tc:
        with tc.tile_pool(name="persist", bufs=1) as persist:

            tri_sb = persist.tile([128, 128], f32r, tag="tri")
            nc.sync.dma_start(out=tri_sb[:], in_=tri[:])

            qT_all = persist.tile([128, 4, T], f32r, tag="qT")   # pair p: rows 0-63 head 2p, 64-127 head 2p+1
            kT_all = persist.tile([128, 4, T], f32r, tag="kT")
            v_all = persist.tile([128, NTK, HPC * 65], f32r, tag="v")  # per head: 64 v cols + 1 ones col

            # ones columns (col 64 of each 65-block); memset can't write f32r,
            # so copy from the fp32 1.0 const AP (copy rounds to f32r)
            ones_ap = v_all[:].rearrange("p t (h e) -> p t h e", h=HPC)[:, :, :, 64:65]
            ones_const = nc.const_aps.tensor(1.0, (128, NTK, HPC, 1), mybir.dt.float32)
            nc.vector.tensor_copy(ones_ap, ones_const)

            # ---------------- QKV projections ----------------
            with tc.tile_pool(name="xw", bufs=1) as xwp, \
                 tc.tile_pool(name="wpool", bufs=2) as wp, \
                 tc.tile_pool(name="acc", bufs=6, space="PSUM") as acc_pool:
                xT_sb = xwp.tile([128, ND, T], f32r, tag="xT")
                for d in range(ND):
                    nc.sync.dma_start(out=xT_sb[:, d, :], in_=xT[128 * d:128 * (d + 1), :])

                w_sb = wp.tile([128, ND, DH], f32r, tag="w")
                for d in range(ND):
                    nc.sync.dma_start(out=w_sb[:, d, :], in_=wv[128 * d:128 * (d + 1), :])
                for tt in range(NTK):
                    ps = acc_pool.tile([128, 512], f32, tag="acc")
                    for d in range(ND):
                        nc.tensor.matmul(
                            ps[:],
                            xT_sb[:, d, 128 * tt:128 * (tt + 1)],
                            w_sb[:, d, :],
                            start=(d == 0), stop=(d == ND - 1),
                        )
                    v_dst = v_all[:].rearrange("p t (h e) -> p t h e", h=HPC)[:, tt, :, 0:64]
                    v_src = ps[:].rearrange("p (h e) -> p h e", h=HPC)
                    nc.scalar.copy(v_dst, v_src)

                # k then q, tq-chunk-major, so attention tq0's inputs (v, kT
                # tiles, qT chunk 0) are all ready early and the exp stream
                # overlaps the remaining projections
                for w_dram, dst in ((wk, kT_all), (wq, qT_all)):
                    w_sb = wp.tile([128, ND, DH], f32r, tag="w")
                    for d in range(ND):
                        nc.sync.dma_start(out=w_sb[:, d, :], in_=w_dram[128 * d:128 * (d + 1), :])
                    for c in range(NTQ):
                        for p in range(4):
                            ps = acc_pool.tile([128, 512], f32, tag="acc")
                            for d in range(ND):
                                nc.tensor.matmul(
                                    ps[:],
                                    w_sb[:, d, 128 * p:128 * (p + 1)],
                                    xT_sb[:, d, TQC * c:TQC * (c + 1)],
                                    start=(d == 0), stop=(d == ND - 1),
                                )
                            nc.scalar.copy(dst[:, p, TQC * c:TQC * (c + 1)], ps[:])

            # ---------------- attention + output projection ----------------
            with tc.tile_pool(name="wo_pool", bufs=1) as wop, \
                 tc.tile_pool(name="probs", bufs=5) as probs_pool, \
                 tc.tile_pool(name="ctxsb", bufs=9) as ctx_sb_pool, \
                 tc.tile_pool(name="recip", bufs=3) as recip_pool, \
                 tc.tile_pool(name="bcast", bufs=3) as bcast_pool, \
                 tc.tile_pool(name="outsb", bufs=4) as out_sb_pool, \
                 tc.tile_pool(name="psc", bufs=3, space="PSUM") as sc_pool, \
                 tc.tile_pool(name="pctx", bufs=2, space="PSUM") as ctx_pool:

                # pair layout: pair p rows 0-63 = head 2p, rows 64-127 = head
                # 2p+1, so out-proj matmuls of a pair auto-row-tile (64-strips)
                wo_sb = wop.tile([128, 4, DOUT], f32r, tag="wo")
                for p in range(4):
                    nc.sync.dma_start(out=wo_sb[:, p, :], in_=wo[128 * p:128 * (p + 1), :])

                def emit_outproj(n, ctx_of):
                    for s_ in range(4):
                        osb = out_sb_pool.tile([128, DOUT], f32, tag="osb")
                        for half in (0, 1):
                            op = sc_pool.tile([128, 1024], f32, tag="sc")
                            op0 = op[:, 0:512]
                            op1 = op[:, 512:1024]
                            for p in range(4):
                                for r, opx in ((0, op0), (1, op1)):
                                    base = 64 * r
                                    nc.tensor.matmul(
                                        opx[:],
                                        ctx_of[p][base:base + 64, 128 * s_:128 * (s_ + 1)],
                                        wo_sb[base:base + 64, p, 512 * half:512 * (half + 1)],
                                        start=(p == 0), stop=(p == 3),
                                    )
                            tmp = out_sb_pool.tile([128, 512], f32, tag="otmp")
                            nc.vector.tensor_copy(tmp[:], op1[:])
                            nc.vector.tensor_add(osb[:, 512 * half:512 * (half + 1)], op0[:], tmp[:])
                        row0 = TQC * n + 128 * s_
                        nc.sync.dma_start(out=out[row0:row0 + 128, :], in_=osb[:])

                prev = None
                for n in range(NTQ):
                    n_tk = 4 * (n + 1)      # causal: tk tiles 0..n_tk-1
                    ctx_of = {}
                    for p in range(4):
                        cs_pair = ctx_sb_pool.tile([128, 512], f32r, tag="cs")
                        ctx_of[p] = cs_pair
                        for r in range(2):
                            h = 2 * p + r
                            base = 64 * r
                            ctx = ctx_pool.tile([65, 512], f32, tag="ctx")
                            n_grp = n_tk // 2

                            def emit_scores(g):
                                sc = sc_pool.tile([128, 1024], f32, tag="sc")
                                pt = probs_pool.tile([128, 1024], f32r, tag="pt")
                                for bank in (0, 1):
                                    tk = 2 * g + bank
                                    nc.tensor.matmul(
                                        sc[:, 512 * bank:512 * (bank + 1)],
                                        kT_all[base:base + 64, p, 128 * tk:128 * (tk + 1)],
                                        qT_all[base:base + 64, p, TQC * n:TQC * (n + 1)],
                                        start=True, stop=True,
                                    )
                                grp_diag = (128 * (2 * g) - TQC * n) > 0
                                if not grp_diag:
                                    # fully-valid group: one bank-aligned exp
                                    # over both banks
                                    nc.scalar.activation(
                                        pt[:, 0:1024], sc[:, 0:1024], EXP, scale=0.125,
                                    )
                                else:
                                    for bank in (0, 1):
                                        o = max(0, 128 * (2 * g + bank) - TQC * n)
                                        c0 = 512 * bank
                                        nc.scalar.activation(
                                            pt[:, c0 + o:c0 + 512], sc[:, c0 + o:c0 + 512],
                                            EXP, scale=0.125,
                                        )
                                return pt

                            def emit_ctx(g, pt):
                                for bank in (0, 1):
                                    tk = 2 * g + bank
                                    off = 128 * tk - TQC * n
                                    o = max(0, off)
                                    c0 = 512 * bank
                                    if off >= 0:
                                        nc.vector.tensor_mul(
                                            pt[:, c0 + o:c0 + o + 128],
                                            pt[:, c0 + o:c0 + o + 128],
                                            tri_sb[:],
                                        )
                                    nc.tensor.matmul(
                                        ctx[0:65, o:512],
                                        v_all[:, tk, 65 * h:65 * h + 65],
                                        pt[:, c0 + o:c0 + 512],
                                        start=(tk == 0), stop=(tk == n_tk - 1),
                                    )

                            # software pipeline: emit scores+exp one group
                            # ahead of the ctx matmuls so the in-order PE
                            # stream never blocks on the exp of the group it
                            # is about to consume
                            prev_pt = emit_scores(0)
                            for g in range(1, n_grp):
                                cur_pt = emit_scores(g)
                                emit_ctx(g - 1, prev_pt)
                                prev_pt = cur_pt
                            emit_ctx(n_grp - 1, prev_pt)
                            # softmax denominators: psum row 64 -> sbuf row 64
                            # (aligned) -> shift to row 0 -> recip -> broadcast
                            # to rows 0-63 (gpsimd broadcast only works from a
                            # base-0 source on HW)
                            rc = recip_pool.tile([65, 1024], f32, tag="rc")
                            # single psum-source partition-shifted copy
                            # (row 64 -> row 0); shaves an op + sem hop off the
                            # normalize chain latency
                            nc.vector.tensor_copy(rc[0:1, 0:512], ctx[64:65, :])
                            nc.vector.reciprocal_approx_fast(out=rc[0:1, 512:1024], in_=rc[0:1, 0:512])
                            bc = bcast_pool.tile([64, 512], f32, tag="bc")
                            nc.gpsimd.partition_broadcast(bc[:, :], rc[0:1, 512:1024])
                            # out-shifted TT: inputs base 0, output rows 64r..
                            nc.vector.tensor_mul(cs_pair[base:base + 64, :], ctx[0:64, :], bc[:, :])

                    # defer this tq's out-projection until after the next
                    # tq's head work: its cs inputs then have a full tq of
                    # slack and the PE never stalls on the normalize chains
                    if prev is not None:
                        emit_outproj(prev[0], prev[1])
                    prev = (n, ctx_of)

                emit_outproj(prev[0], prev[1])
    nc.compile()
    return nc


def _in_maps(x, W_query, W_key, W_value, W_out):
    tri_np = np.triu(np.ones((128, 128), dtype=np.float32))
    maps = []
    for c in range(NCORES):
        b, g = divmod(c, 2)
        maps.append({
            "xT": np.ascontiguousarray(np.asarray(x[b], dtype=np.float32).T),
            "wq": np.ascontiguousarray(np.asarray(W_query, dtype=np.float32)[:, DH * g:DH * (g + 1)]),
            "wk": np.ascontiguousarray(np.asarray(W_key, dtype=np.float32)[:, DH * g:DH * (g + 1)]),
            "wv": np.ascontiguousarray(np.asarray(W_value, dtype=np.float32)[:, DH * g:DH * (g + 1)]),
            "wo": np.ascontiguousarray(np.asarray(W_out, dtype=np.float32)[DH * g:DH * (g + 1), :]),
            "tri": tri_np,
        })
    return maps


def _run(in_maps, trace=False):
    from concourse.bass_utils import run_bass_kernel_spmd

    global _built
    if _built is None:
        _built = _build()
    return run_bass_kernel_spmd(_built, in_maps, list(range(NCORES)), trace=trace)


def kernel(x, W_query, W_key, W_value, W_out, b_out):
    res = _run(_in_maps(x, W_query, W_key, W_value, W_out))
    bias = np.asarray(b_out, dtype=np.float32)
    out = np.empty((B, T, DOUT), dtype=np.float32)
    for b in range(B):
        out[b] = res.results[2 * b]["out"] + res.results[2 * b + 1]["out"] + bias[None, :]
    return out
